# revision 19
# baseline (speedup 1.0000x reference)
"""Contrastive loss (video/audio) Trainium2 Bass kernel.

Full inputs: video [64,512,512] f32, audio [64,512,512] f32, mask [64,512] i32.
Data-parallel over batch: 8 cores x 8 batch elements. Each core computes its
partial loss sum on device; host adds the 8 scalars and divides by B.

Per-core pipeline (v5, bf16 data / fp32 accumulation):
  mask -> score -> one-hot indicator (natural layout) -> PE-transposed to
  T-partitioned layout.  Anchor rows are extracted AND broadcast in one step:
  abc[b] = sum_c (ind_t[:,(c,b)] bcast as lhsT) @ chunk_c  -- 4 accumulating
  matmuls per (b, modality), PSUM [128,512].  No indirect DMA.
  Main loop per (b,c): r = row sum-of-squares (ACT Square+accum, a slice on
  DVE STT for balance); s_raw = DVE STT(chunk * abc, accum).
  Anchor norms and the positive logit are recovered from the r/s accumulator
  tiles with the same indicator (elementwise mult + PE ones-matvec), so the
  whole normalization/exp/log tail runs on [1,8] partition-0 rows.
"""

import numpy as np
from contextlib import ExitStack

import concourse.bass as bass
import concourse.tile as tile
from concourse import mybir
from concourse.bass_utils import run_bass_kernel_spmd

F32 = mybir.dt.float32
BF16 = mybir.dt.bfloat16
I32 = mybir.dt.int32
AF = mybir.ActivationFunctionType
OP = mybir.AluOpType
AX = mybir.AxisListType

B, T, D = 64, 512, 512
NCORES = 8
BL = B // NCORES          # 8 batch elements per core
P = 128                   # partitions
C = T // P                # 4 T-chunks per matrix
TEMP = 0.07
USE_BF16 = True
R_ON_DVE = 10             # of the 64 r-square chunks, this many run on DVE


def build_kernel(ctx: ExitStack, tc: tile.TileContext, video, audio, mask, out):
    nc = tc.nc
    DT = BF16 if USE_BF16 else F32

    persist = ctx.enter_context(tc.tile_pool(name="persist", bufs=1))
    data = ctx.enter_context(tc.tile_pool(name="data", bufs=3))
    scr = ctx.enter_context(tc.tile_pool(name="scr", bufs=2))
    psum = ctx.enter_context(tc.tile_pool(name="psum", bufs=2, space="PSUM"))
    psum1 = ctx.enter_context(tc.tile_pool(name="psum1", bufs=1, space="PSUM"))

    # ---------------- data loads (issue first: mask, then b tiles) ---------
    mask_i = persist.tile([BL, T], I32, tag="mask_i")
    nc.sync.dma_start(mask_i[:], mask[:, :])

    # block tiling: t = c*128 + p  (matches PE-transposed indicator blocks)
    vid_r = video.rearrange("b (c p) d -> b p c d", p=P)   # [8,128,4,512]
    aud_r = audio.rearrange("b (c p) d -> b p c d", p=P)
    vts, ats = [], []
    for b in range(BL):
        at = data.tile([P, C * D], DT, tag="aud")
        nc.sync.dma_start(at[:].rearrange("p (c d) -> p c d", d=D), aud_r[b])
        vt = data.tile([P, C * D], DT, tag="vid")
        nc.sync.dma_start(vt[:].rearrange("p (c d) -> p c d", d=D), vid_r[b])
        vts.append(vt)
        ats.append(at)

    # ---------------- indicator: natural then T-partitioned ----------------
    mask_f = persist.tile([BL, T], F32, tag="mask_f")
    nc.vector.tensor_copy(mask_f[:], mask_i[:])
    iota_i = persist.tile([BL, T], I32, tag="iota_i")
    nc.gpsimd.iota(iota_i[:], pattern=[[1, T]], base=0, channel_multiplier=0)
    iota_f = persist.tile([BL, T], F32, tag="iota_f")
    nc.vector.tensor_copy(iota_f[:], iota_i[:])
    score = persist.tile([BL, T], F32, tag="score")
    nc.vector.scalar_tensor_tensor(
        out=score[:], in0=mask_f[:], scalar=1024.0, in1=iota_f[:],
        op0=OP.mult, op1=OP.subtract)
    maxs = persist.tile([BL, 1], F32, tag="maxs")
    nc.vector.reduce_max(maxs[:], score[:], axis=AX.X)
    ind_nat = persist.tile([BL, T], DT, tag="ind_nat")   # one-hot, exact 0/1
    nc.vector.tensor_scalar(out=ind_nat[:], in0=score[:],
                            scalar1=maxs[:, :1], scalar2=None,
                            op0=OP.is_equal)
    # 8x8 identity for the PE transpose
    eyei = persist.tile([BL, BL], I32, tag="eyei")
    nc.gpsimd.iota(eyei[:], pattern=[[1, BL]], base=0, channel_multiplier=-1)
    eyez = persist.tile([BL, BL], I32, tag="eyez")
    nc.vector.tensor_scalar(out=eyez[:], in0=eyei[:], scalar1=0,
                            scalar2=None, op0=OP.is_equal)
    eyef = persist.tile([BL, BL], DT, tag="eyef")
    nc.vector.tensor_copy(eyef[:], eyez[:])
    # transpose each [8,128] block -> [128,8]; ind_t cols are (c, b)
    ind_t = persist.tile([P, C * BL], DT, tag="ind_t")
    for c in range(C):
        tp = psum1.tile([P, BL], DT, tag="tp")
        nc.tensor.transpose(out=tp[:], in_=ind_nat[:, c * P:(c + 1) * P],
                            identity=eyef[:])
        nc.vector.tensor_copy(ind_t[:, c * BL:c * BL + BL], tp[:])

    # ---------------- main loop ---------------------------------------------
    rv_t = persist.tile([P, BL * C], F32, tag="rv_t")   # ||video_t||^2
    ra_t = persist.tile([P, BL * C], F32, tag="ra_t")   # ||audio_t||^2
    sa_t = persist.tile([P, BL * C], F32, tag="sa_t")   # video_t . anc_a(raw)
    sv_t = persist.tile([P, BL * C], F32, tag="sv_t")   # audio_t . anc_v(raw)

    for b in range(BL):
        vt, at = vts[b], ats[b]
        # anchor extraction fused with broadcast:
        # abc[m,n] = sum_c sum_p ind_t[p,(c,b)] * audio[t(c,p), n]
        abc = psum.tile([P, D], F32, tag="abc")
        vbc = psum.tile([P, D], F32, tag="vbc")
        for c in range(C):
            sel = ind_t[:, c * BL + b:c * BL + b + 1].to_broadcast([P, P])
            nc.tensor.matmul(out=abc[:], lhsT=sel,
                             rhs=at[:, c * D:(c + 1) * D],
                             start=(c == 0), stop=(c == C - 1))
        for c in range(C):
            sel = ind_t[:, c * BL + b:c * BL + b + 1].to_broadcast([P, P])
            nc.tensor.matmul(out=vbc[:], lhsT=sel,
                             rhs=vt[:, c * D:(c + 1) * D],
                             start=(c == 0), stop=(c == C - 1))
        # r first (keeps ACT/DVE streams unblocked), then s
        for c in range(C):
            col = b * C + c
            vch = vt[:, c * D:(c + 1) * D]
            ach = at[:, c * D:(c + 1) * D]
            if b == 0:
                r1 = scr.tile([P, D], DT, tag="r1d")
                nc.vector.scalar_tensor_tensor(
                    out=r1[:], in0=vch, scalar=1.0, in1=vch,
                    op0=OP.mult, op1=OP.mult,
                    accum_out=rv_t[:, col:col + 1])
            else:
                r1 = scr.tile([P, D], DT, tag="r1a")
                nc.scalar.activation(r1[:], vch, AF.Square,
                                     accum_out=rv_t[:, col:col + 1])
            if b == BL - 1:
                r2 = scr.tile([P, D], DT, tag="r2d")
                nc.vector.scalar_tensor_tensor(
                    out=r2[:], in0=ach, scalar=1.0, in1=ach,
                    op0=OP.mult, op1=OP.mult,
                    accum_out=ra_t[:, col:col + 1])
            else:
                r2 = scr.tile([P, D], DT, tag="r2a")
                nc.scalar.activation(r2[:], ach, AF.Square,
                                     accum_out=ra_t[:, col:col + 1])
        for c in range(C):
            col = b * C + c
            vch = vt[:, c * D:(c + 1) * D]
            ach = at[:, c * D:(c + 1) * D]
            s1 = scr.tile([P, D], DT, tag="s1")
            nc.vector.scalar_tensor_tensor(
                out=s1[:], in0=vch, scalar=1.0, in1=abc[:],
                op0=OP.mult, op1=OP.mult, accum_out=sa_t[:, col:col + 1])
            s2 = scr.tile([P, D], DT, tag="s2")
            nc.vector.scalar_tensor_tensor(
                out=s2[:], in0=ach, scalar=1.0, in1=vbc[:],
                op0=OP.mult, op1=OP.mult, accum_out=sv_t[:, col:col + 1])

    # ------- anchor norms + pos from accumulator tiles via indicator -------
    # ind_t cols are (c,b); accumulator cols are (b,c): use a strided view.
    ind_bc = ind_t[:].rearrange("p (c b) -> p c b", b=BL) \
        .rearrange("p c b -> p b c")                       # [128, b, c] view
    prod = persist.tile([P, 3 * BL * C], F32, tag="prod")
    pv = prod[:, 0:BL * C].rearrange("p (b c) -> p b c", c=C)
    pa = prod[:, BL * C:2 * BL * C].rearrange("p (b c) -> p b c", c=C)
    pp = prod[:, 2 * BL * C:3 * BL * C].rearrange("p (b c) -> p b c", c=C)
    nc.vector.tensor_tensor(pv, rv_t[:].rearrange("p (b c) -> p b c", c=C),
                            ind_bc, op=OP.mult)
    nc.vector.tensor_tensor(pa, ra_t[:].rearrange("p (b c) -> p b c", c=C),
                            ind_bc, op=OP.mult)
    nc.vector.tensor_tensor(pp, sa_t[:].rearrange("p (b c) -> p b c", c=C),
                            ind_bc, op=OP.mult)
    ones_col = persist.tile([P, 1], F32, tag="ones_col")
    nc.vector.memset(ones_col[:], 1.0)
    psel = psum1.tile([1, 3 * BL * C], F32, tag="psel")
    nc.tensor.matmul(out=psel[:], lhsT=ones_col[:], rhs=prod[:],
                     start=True, stop=True)
    rav_row = persist.tile([1, BL], F32, tag="rav_row")   # ||anc_v||^2
    nc.vector.reduce_sum(
        rav_row[:], psel[:, 0:BL * C].rearrange("p (b c) -> p b c", c=C),
        axis=AX.X)
    raa_row = persist.tile([1, BL], F32, tag="raa_row")   # ||anc_a||^2
    nc.vector.reduce_sum(
        raa_row[:],
        psel[:, BL * C:2 * BL * C].rearrange("p (b c) -> p b c", c=C),
        axis=AX.X)
    pod_row = persist.tile([1, BL], F32, tag="pod_row")   # anc_v . anc_a
    nc.vector.reduce_sum(
        pod_row[:],
        psel[:, 2 * BL * C:3 * BL * C].rearrange("p (b c) -> p b c", c=C),
        axis=AX.X)
    sq_ra = persist.tile([1, BL], F32, tag="sq_ra")
    nc.scalar.activation(sq_ra[:], raa_row[:], AF.Sqrt, scale=TEMP * TEMP)
    inva_row = persist.tile([1, BL], F32, tag="inva_row")
    nc.vector.reciprocal(inva_row[:], sq_ra[:])
    sq_rv = persist.tile([1, BL], F32, tag="sq_rv")
    nc.scalar.activation(sq_rv[:], rav_row[:], AF.Sqrt, scale=TEMP * TEMP)
    invv_row = persist.tile([1, BL], F32, tag="invv_row")
    nc.vector.reciprocal(invv_row[:], sq_rv[:])
    pos_row = persist.tile([1, BL], F32, tag="pos_row")
    nc.vector.tensor_tensor(pos_row[:], pod_row[:], inva_row[:], op=OP.mult)
    nc.vector.tensor_tensor(pos_row[:], pos_row[:], invv_row[:], op=OP.mult)
    nc.vector.tensor_scalar_mul(pos_row[:], pos_row[:], TEMP)
    # broadcast inv rows to [128, 8] via PE outer
    ones_row = persist.tile([1, P], F32, tag="ones_row")
    nc.vector.memset(ones_row[:], 1.0)
    inv_bc = psum1.tile([P, 2 * BL], F32, tag="inv_bc")
    nc.tensor.matmul(out=inv_bc[:, 0:BL], lhsT=ones_row[:], rhs=inva_row[:],
                     start=True, stop=True)
    nc.tensor.matmul(out=inv_bc[:, BL:2 * BL], lhsT=ones_row[:],
                     rhs=invv_row[:], start=True, stop=True)
    inva_bc = inv_bc[:, 0:BL]
    invv_bc = inv_bc[:, BL:2 * BL]

    # ---------------- post: scale, exp, reduce, combine --------------------
    srt_v = persist.tile([P, BL * C], F32, tag="srt_v")
    nc.scalar.activation(srt_v[:], rv_t[:], AF.Sqrt)
    irt_v = persist.tile([P, BL * C], F32, tag="irt_v")
    nc.vector.reciprocal(irt_v[:], srt_v[:])
    srt_a = persist.tile([P, BL * C], F32, tag="srt_a")
    nc.scalar.activation(srt_a[:], ra_t[:], AF.Sqrt)
    irt_a = persist.tile([P, BL * C], F32, tag="irt_a")
    nc.vector.reciprocal(irt_a[:], srt_a[:])

    cmb_a = persist.tile([P, BL, C], F32, tag="cmb_a")
    nc.vector.tensor_tensor(
        cmb_a[:], irt_v[:].rearrange("p (a b) -> p a b", b=C),
        inva_bc.to_broadcast([P, BL, C]), op=OP.mult)
    cmb_v = persist.tile([P, BL, C], F32, tag="cmb_v")
    nc.vector.tensor_tensor(
        cmb_v[:], irt_a[:].rearrange("p (a b) -> p a b", b=C),
        invv_bc.to_broadcast([P, BL, C]), op=OP.mult)

    ssc_a = persist.tile([P, BL * C], F32, tag="ssc_a")
    nc.vector.tensor_tensor(ssc_a[:], sa_t[:],
                            cmb_a[:].rearrange("p a b -> p (a b)"), op=OP.mult)
    ssc_v = persist.tile([P, BL * C], F32, tag="ssc_v")
    nc.vector.tensor_tensor(ssc_v[:], sv_t[:],
                            cmb_v[:].rearrange("p a b -> p (a b)"), op=OP.mult)

    exp_a = persist.tile([P, BL * C], F32, tag="exp_a")
    nc.scalar.activation(exp_a[:], ssc_a[:], AF.Exp)
    exp_v = persist.tile([P, BL * C], F32, tag="exp_v")
    nc.scalar.activation(exp_v[:], ssc_v[:], AF.Exp)

    pex = psum1.tile([1, 2 * BL * C], F32, tag="pex")
    nc.tensor.matmul(out=pex[:, 0:BL * C], lhsT=ones_col[:], rhs=exp_a[:],
                     start=True, stop=True)
    nc.tensor.matmul(out=pex[:, BL * C:2 * BL * C], lhsT=ones_col[:],
                     rhs=exp_v[:], start=True, stop=True)

    se_a = persist.tile([1, BL], F32, tag="se_a")
    nc.vector.reduce_sum(
        se_a[:], pex[:, 0:BL * C].rearrange("p (a b) -> p a b", b=C),
        axis=AX.X)
    se_v = persist.tile([1, BL], F32, tag="se_v")
    nc.vector.reduce_sum(
        se_v[:], pex[:, BL * C:2 * BL * C].rearrange("p (a b) -> p a b", b=C),
        axis=AX.X)

    epos = persist.tile([1, BL], F32, tag="epos")
    nc.scalar.activation(epos[:], pos_row[:], AF.Exp)
    neg_a = persist.tile([1, BL], F32, tag="neg_a")
    nc.vector.tensor_tensor(neg_a[:], se_a[:], epos[:], op=OP.subtract)
    neg_v = persist.tile([1, BL], F32, tag="neg_v")
    nc.vector.tensor_tensor(neg_v[:], se_v[:], epos[:], op=OP.subtract)
    lg_a = persist.tile([1, BL], F32, tag="lg_a")
    nc.scalar.activation(lg_a[:], neg_a[:], AF.Ln)
    lg_v = persist.tile([1, BL], F32, tag="lg_v")
    nc.scalar.activation(lg_v[:], neg_v[:], AF.Ln)
    term = persist.tile([1, BL], F32, tag="term")
    nc.vector.tensor_tensor(term[:], lg_a[:], lg_v[:], op=OP.add)
    nc.vector.tensor_scalar_mul(term[:], term[:], 0.5)
    nc.vector.tensor_tensor(term[:], term[:], pos_row[:], op=OP.subtract)
    tot = persist.tile([1, 1], F32, tag="tot")
    nc.vector.reduce_sum(tot[:], term[:], axis=AX.X)
    nc.sync.dma_start(out[:, :], tot[:])



# ---------------------------------------------------------------------------
# BIR legalization for this walrus build:
#  - it rejects instructions carrying more than one semaphore wait
#    ("Too many sync wait commands"): hoist extra waits onto single-wait
#    NoOp carriers on the same engine.
#  - the Tile tail's EVENT_SEMAPHORE_RANGE_CLEAR raw-ISA encoding mismatches
#    ("ISA wrong length"): replace with a sem-resetting Drain and drop the
#    trailing barrier that only fences the reset.
_LEGALIZE_N = [0]


def _legalize(nc):
    for fn in nc.m.functions:
        for bb in fn.blocks:
            new = []
            tail_trim = False
            for inst in bb.instructions:
                si = inst.sync_info
                if si is not None and si.on_wait and len(si.on_wait) > 1:
                    for w in list(si.on_wait[:-1]):
                        _LEGALIZE_N[0] += 1
                        new.append(mybir.InstNoOp(
                            name=f"I-lw{_LEGALIZE_N[0]}",
                            opcode="NoOp",
                            engine=inst.engine,
                            sync_info=mybir.SyncInfo(on_wait=[w],
                                                     on_update=[]),
                        ))
                    si.on_wait = [si.on_wait[-1]]
                if (isinstance(inst, mybir.InstISA)
                        and getattr(inst, "op_name", "") ==
                        "EVENT_SEMAPHORE_RANGE_CLEAR"):
                    ad = getattr(inst, "ant_dict", None) or {}
                    _LEGALIZE_N[0] += 1
                    new.append(mybir.InstDrain(
                        name=f"I-lc{_LEGALIZE_N[0]}",
                        opcode="Drain",
                        engine=inst.engine,
                        is_reset_sema=True,
                        reset_range_start=ad.get("range_first"),
                        reset_range_stop=ad.get("range_last"),
                    ))
                    tail_trim = True
                    continue
                if tail_trim and inst.opcode in ("EventSemaphore", "Drain"):
                    continue
                new.append(inst)
            bb.instructions[:] = new


_CACHE = {}


def _get_nc():
    if "nc" not in _CACHE:
        nc = bass.Bass("TRN2", target_bir_lowering=False, debug=False,
                       num_devices=NCORES)
        dt = BF16 if USE_BF16 else F32
        video = nc.dram_tensor("video", [BL, T, D], dt,
                               kind="ExternalInput").ap()
        audio = nc.dram_tensor("audio", [BL, T, D], dt,
                               kind="ExternalInput").ap()
        mask = nc.dram_tensor("mask", [BL, T], I32, kind="ExternalInput").ap()
        out = nc.dram_tensor("out", [1, 1], F32, kind="ExternalOutput").ap()
        with tile.TileContext(nc) as tc:
            with ExitStack() as ctx:
                build_kernel(ctx, tc, video, audio, mask, out)
        _legalize(nc)
        _CACHE["nc"] = nc
    return _CACHE["nc"]


def kernel(video, audio, mask, _want_results=False):
    import ml_dtypes
    ddt = ml_dtypes.bfloat16 if USE_BF16 else np.float32
    video = np.ascontiguousarray(np.asarray(video).astype(ddt))
    audio = np.ascontiguousarray(np.asarray(audio).astype(ddt))
    mask = np.ascontiguousarray(np.asarray(mask, dtype=np.int32))
    nc = _get_nc()
    in_maps = []
    for i in range(NCORES):
        sl = slice(i * BL, (i + 1) * BL)
        in_maps.append({"video": video[sl], "audio": audio[sl],
                        "mask": mask[sl]})
    res = run_bass_kernel_spmd(nc, in_maps, list(range(NCORES)))
    parts = [res.results[i]["out"][0, 0] for i in range(NCORES)]
    loss = np.float32(np.sum(np.asarray(parts, dtype=np.float64)) / B)
    outarr = np.asarray([loss], dtype=np.float32)
    if _want_results:
        return outarr, res
    return outarr


# revision 21
# speedup vs baseline: 1.0447x; 1.0447x over previous
"""Contrastive loss (video/audio) Trainium2 Bass kernel.

Full inputs: video [64,512,512] f32, audio [64,512,512] f32, mask [64,512] i32.
Data-parallel over batch: 8 cores x 8 batch elements. Each core computes its
partial loss sum on device; host adds the 8 scalars and divides by B.

Per-core pipeline (v5, bf16 data / fp32 accumulation):
  mask -> score -> one-hot indicator (natural layout) -> PE-transposed to
  T-partitioned layout.  Anchor rows are extracted AND broadcast in one step:
  abc[b] = sum_c (ind_t[:,(c,b)] bcast as lhsT) @ chunk_c  -- 4 accumulating
  matmuls per (b, modality), PSUM [128,512].  No indirect DMA.
  Main loop per (b,c): r = row sum-of-squares (ACT Square+accum, a slice on
  DVE STT for balance); s_raw = DVE STT(chunk * abc, accum).
  Anchor norms and the positive logit are recovered from the r/s accumulator
  tiles with the same indicator (elementwise mult + PE ones-matvec), so the
  whole normalization/exp/log tail runs on [1,8] partition-0 rows.
"""

import numpy as np
from contextlib import ExitStack

import concourse.bass as bass
import concourse.tile as tile
from concourse import mybir
from concourse.bass_utils import run_bass_kernel_spmd

F32 = mybir.dt.float32
BF16 = mybir.dt.bfloat16
I32 = mybir.dt.int32
AF = mybir.ActivationFunctionType
OP = mybir.AluOpType
AX = mybir.AxisListType

B, T, D = 64, 512, 512
NCORES = 8
BL = B // NCORES          # 8 batch elements per core
P = 128                   # partitions
C = T // P                # 4 T-chunks per matrix
TEMP = 0.07
USE_BF16 = True
R_ON_DVE = 10             # of the 64 r-square chunks, this many run on DVE


def build_kernel(ctx: ExitStack, tc: tile.TileContext, video, audio, mask, out):
    nc = tc.nc
    DT = BF16 if USE_BF16 else F32

    persist = ctx.enter_context(tc.tile_pool(name="persist", bufs=1))
    data = ctx.enter_context(tc.tile_pool(name="data", bufs=3))
    scr = ctx.enter_context(tc.tile_pool(name="scr", bufs=2))
    psum = ctx.enter_context(tc.tile_pool(name="psum", bufs=2, space="PSUM"))
    psum1 = ctx.enter_context(tc.tile_pool(name="psum1", bufs=1, space="PSUM"))

    # ---------------- data loads (issue first: mask, then b tiles) ---------
    mask_i = persist.tile([BL, T], I32, tag="mask_i")
    nc.sync.dma_start(mask_i[:], mask[:, :])

    # block tiling: t = c*128 + p  (matches PE-transposed indicator blocks)
    vid_r = video.rearrange("b (c p) d -> b p c d", p=P)   # [8,128,4,512]
    aud_r = audio.rearrange("b (c p) d -> b p c d", p=P)
    vts, ats = [], []
    for b in range(BL):
        at = data.tile([P, C * D], DT, tag="aud")
        nc.sync.dma_start(at[:].rearrange("p (c d) -> p c d", d=D), aud_r[b])
        vt = data.tile([P, C * D], DT, tag="vid")
        nc.sync.dma_start(vt[:].rearrange("p (c d) -> p c d", d=D), vid_r[b])
        vts.append(vt)
        ats.append(at)

    # ---------------- indicator: natural then T-partitioned ----------------
    mask_f = persist.tile([BL, T], F32, tag="mask_f")
    nc.vector.tensor_copy(mask_f[:], mask_i[:])
    iota_i = persist.tile([BL, T], I32, tag="iota_i")
    nc.gpsimd.iota(iota_i[:], pattern=[[1, T]], base=0, channel_multiplier=0)
    iota_f = persist.tile([BL, T], F32, tag="iota_f")
    nc.vector.tensor_copy(iota_f[:], iota_i[:])
    score = persist.tile([BL, T], F32, tag="score")
    nc.vector.scalar_tensor_tensor(
        out=score[:], in0=mask_f[:], scalar=1024.0, in1=iota_f[:],
        op0=OP.mult, op1=OP.subtract)
    maxs = persist.tile([BL, 1], F32, tag="maxs")
    nc.vector.reduce_max(maxs[:], score[:], axis=AX.X)
    ind_nat = persist.tile([BL, T], DT, tag="ind_nat")   # one-hot, exact 0/1
    nc.vector.tensor_scalar(out=ind_nat[:], in0=score[:],
                            scalar1=maxs[:, :1], scalar2=None,
                            op0=OP.is_equal)
    # 8x8 identity for the PE transpose
    eyei = persist.tile([BL, BL], I32, tag="eyei")
    nc.gpsimd.iota(eyei[:], pattern=[[1, BL]], base=0, channel_multiplier=-1)
    eyez = persist.tile([BL, BL], I32, tag="eyez")
    nc.vector.tensor_scalar(out=eyez[:], in0=eyei[:], scalar1=0,
                            scalar2=None, op0=OP.is_equal)
    eyef = persist.tile([BL, BL], DT, tag="eyef")
    nc.vector.tensor_copy(eyef[:], eyez[:])
    # transpose each [8,128] block -> [128,8]; ind_t cols are (c, b)
    ind_t = persist.tile([P, C * BL], DT, tag="ind_t")
    for c in range(C):
        tp = psum1.tile([P, BL], DT, tag="tp")
        nc.tensor.transpose(out=tp[:], in_=ind_nat[:, c * P:(c + 1) * P],
                            identity=eyef[:])
        nc.vector.tensor_copy(ind_t[:, c * BL:c * BL + BL], tp[:])

    # ---------------- main loop ---------------------------------------------
    rv_t = persist.tile([P, BL * C], F32, tag="rv_t")   # ||video_t||^2
    ra_t = persist.tile([P, BL * C], F32, tag="ra_t")   # ||audio_t||^2
    sa_t = persist.tile([P, BL * C], F32, tag="sa_t")   # video_t . anc_a(raw)
    sv_t = persist.tile([P, BL * C], F32, tag="sv_t")   # audio_t . anc_v(raw)

    for b in range(BL):
        vt, at = vts[b], ats[b]
        # anchor extraction fused with broadcast:
        # abc[m,n] = sum_c sum_p ind_t[p,(c,b)] * audio[t(c,p), n]
        abc = psum.tile([P, D], F32, tag="abc")
        vbc = psum.tile([P, D], F32, tag="vbc")
        for c in range(C):
            sel = ind_t[:, c * BL + b:c * BL + b + 1].to_broadcast([P, P])
            nc.tensor.matmul(out=abc[:], lhsT=sel,
                             rhs=at[:, c * D:(c + 1) * D],
                             start=(c == 0), stop=(c == C - 1))
        for c in range(C):
            sel = ind_t[:, c * BL + b:c * BL + b + 1].to_broadcast([P, P])
            nc.tensor.matmul(out=vbc[:], lhsT=sel,
                             rhs=vt[:, c * D:(c + 1) * D],
                             start=(c == 0), stop=(c == C - 1))
        # r first (keeps ACT/DVE streams unblocked), then s
        for c in range(C):
            col = b * C + c
            vch = vt[:, c * D:(c + 1) * D]
            ach = at[:, c * D:(c + 1) * D]
            if b == 0:
                r1 = scr.tile([P, D], DT, tag="r1d")
                nc.vector.scalar_tensor_tensor(
                    out=r1[:], in0=vch, scalar=1.0, in1=vch,
                    op0=OP.mult, op1=OP.mult,
                    accum_out=rv_t[:, col:col + 1])
            else:
                r1 = scr.tile([P, D], DT, tag="r1a")
                nc.scalar.activation(r1[:], vch, AF.Square,
                                     accum_out=rv_t[:, col:col + 1])
            if b == BL - 1:
                r2 = scr.tile([P, D], DT, tag="r2d")
                nc.vector.scalar_tensor_tensor(
                    out=r2[:], in0=ach, scalar=1.0, in1=ach,
                    op0=OP.mult, op1=OP.mult,
                    accum_out=ra_t[:, col:col + 1])
            else:
                r2 = scr.tile([P, D], DT, tag="r2a")
                nc.scalar.activation(r2[:], ach, AF.Square,
                                     accum_out=ra_t[:, col:col + 1])
        for c in range(C):
            col = b * C + c
            vch = vt[:, c * D:(c + 1) * D]
            ach = at[:, c * D:(c + 1) * D]
            s1 = scr.tile([P, D], DT, tag="s1")
            nc.vector.scalar_tensor_tensor(
                out=s1[:], in0=vch, scalar=1.0, in1=abc[:],
                op0=OP.mult, op1=OP.mult, accum_out=sa_t[:, col:col + 1])
            s2 = scr.tile([P, D], DT, tag="s2")
            nc.vector.scalar_tensor_tensor(
                out=s2[:], in0=ach, scalar=1.0, in1=vbc[:],
                op0=OP.mult, op1=OP.mult, accum_out=sv_t[:, col:col + 1])

    # ------- anchor norms + pos from accumulator tiles via indicator -------
    # ind_t cols are (c,b); accumulator cols are (b,c): use a strided view.
    ind_bc = ind_t[:].rearrange("p (c b) -> p c b", b=BL) \
        .rearrange("p c b -> p b c")                       # [128, b, c] view
    prod = persist.tile([P, 3 * BL * C], F32, tag="prod")
    pv = prod[:, 0:BL * C].rearrange("p (b c) -> p b c", c=C)
    pa = prod[:, BL * C:2 * BL * C].rearrange("p (b c) -> p b c", c=C)
    pp = prod[:, 2 * BL * C:3 * BL * C].rearrange("p (b c) -> p b c", c=C)
    nc.vector.tensor_tensor(pv, rv_t[:].rearrange("p (b c) -> p b c", c=C),
                            ind_bc, op=OP.mult)
    nc.vector.tensor_tensor(pa, ra_t[:].rearrange("p (b c) -> p b c", c=C),
                            ind_bc, op=OP.mult)
    nc.vector.tensor_tensor(pp, sa_t[:].rearrange("p (b c) -> p b c", c=C),
                            ind_bc, op=OP.mult)
    ones_col = persist.tile([P, 1], F32, tag="ones_col")
    nc.vector.memset(ones_col[:], 1.0)
    psel = psum1.tile([1, 3 * BL * C], F32, tag="psel")
    nc.tensor.matmul(out=psel[:], lhsT=ones_col[:], rhs=prod[:],
                     start=True, stop=True)
    rav_row = persist.tile([1, BL], F32, tag="rav_row")   # ||anc_v||^2
    nc.vector.reduce_sum(
        rav_row[:], psel[:, 0:BL * C].rearrange("p (b c) -> p b c", c=C),
        axis=AX.X)
    raa_row = persist.tile([1, BL], F32, tag="raa_row")   # ||anc_a||^2
    nc.vector.reduce_sum(
        raa_row[:],
        psel[:, BL * C:2 * BL * C].rearrange("p (b c) -> p b c", c=C),
        axis=AX.X)
    pod_row = persist.tile([1, BL], F32, tag="pod_row")   # anc_v . anc_a
    nc.vector.reduce_sum(
        pod_row[:],
        psel[:, 2 * BL * C:3 * BL * C].rearrange("p (b c) -> p b c", c=C),
        axis=AX.X)
    sq_ra = persist.tile([1, BL], F32, tag="sq_ra")
    nc.scalar.activation(sq_ra[:], raa_row[:], AF.Sqrt, scale=TEMP * TEMP)
    inva_row = persist.tile([1, BL], F32, tag="inva_row")
    nc.vector.reciprocal(inva_row[:], sq_ra[:])
    sq_rv = persist.tile([1, BL], F32, tag="sq_rv")
    nc.scalar.activation(sq_rv[:], rav_row[:], AF.Sqrt, scale=TEMP * TEMP)
    invv_row = persist.tile([1, BL], F32, tag="invv_row")
    nc.vector.reciprocal(invv_row[:], sq_rv[:])
    pos_row = persist.tile([1, BL], F32, tag="pos_row")
    nc.vector.tensor_tensor(pos_row[:], pod_row[:], inva_row[:], op=OP.mult)
    nc.vector.tensor_tensor(pos_row[:], pos_row[:], invv_row[:], op=OP.mult)
    nc.vector.tensor_scalar_mul(pos_row[:], pos_row[:], TEMP)
    # broadcast inv rows to [128, 8] via PE outer
    ones_row = persist.tile([1, P], F32, tag="ones_row")
    nc.vector.memset(ones_row[:], 1.0)
    inv_bc = psum1.tile([P, 2 * BL], F32, tag="inv_bc")
    nc.tensor.matmul(out=inv_bc[:, 0:BL], lhsT=ones_row[:], rhs=inva_row[:],
                     start=True, stop=True)
    nc.tensor.matmul(out=inv_bc[:, BL:2 * BL], lhsT=ones_row[:],
                     rhs=invv_row[:], start=True, stop=True)
    inva_bc = inv_bc[:, 0:BL]
    invv_bc = inv_bc[:, BL:2 * BL]

    # ---------------- post: scale, exp, reduce, combine --------------------
    srt_v = persist.tile([P, BL * C], F32, tag="srt_v")
    nc.scalar.activation(srt_v[:], rv_t[:], AF.Sqrt)
    irt_v = persist.tile([P, BL * C], F32, tag="irt_v")
    nc.vector.reciprocal(irt_v[:], srt_v[:])
    srt_a = persist.tile([P, BL * C], F32, tag="srt_a")
    nc.scalar.activation(srt_a[:], ra_t[:], AF.Sqrt)
    irt_a = persist.tile([P, BL * C], F32, tag="irt_a")
    nc.vector.reciprocal(irt_a[:], srt_a[:])

    cmb_a = persist.tile([P, BL, C], F32, tag="cmb_a")
    nc.vector.tensor_tensor(
        cmb_a[:], irt_v[:].rearrange("p (a b) -> p a b", b=C),
        inva_bc.to_broadcast([P, BL, C]), op=OP.mult)
    cmb_v = persist.tile([P, BL, C], F32, tag="cmb_v")
    nc.vector.tensor_tensor(
        cmb_v[:], irt_a[:].rearrange("p (a b) -> p a b", b=C),
        invv_bc.to_broadcast([P, BL, C]), op=OP.mult)

    ssc_a = persist.tile([P, BL * C], F32, tag="ssc_a")
    nc.vector.tensor_tensor(ssc_a[:], sa_t[:],
                            cmb_a[:].rearrange("p a b -> p (a b)"), op=OP.mult)
    ssc_v = persist.tile([P, BL * C], F32, tag="ssc_v")
    nc.vector.tensor_tensor(ssc_v[:], sv_t[:],
                            cmb_v[:].rearrange("p a b -> p (a b)"), op=OP.mult)

    exp_a = persist.tile([P, BL * C], F32, tag="exp_a")
    nc.scalar.activation(exp_a[:], ssc_a[:], AF.Exp)
    exp_v = persist.tile([P, BL * C], F32, tag="exp_v")
    nc.scalar.activation(exp_v[:], ssc_v[:], AF.Exp)

    pex = psum1.tile([1, 2 * BL * C], F32, tag="pex")
    nc.tensor.matmul(out=pex[:, 0:BL * C], lhsT=ones_col[:], rhs=exp_a[:],
                     start=True, stop=True)
    nc.tensor.matmul(out=pex[:, BL * C:2 * BL * C], lhsT=ones_col[:],
                     rhs=exp_v[:], start=True, stop=True)

    se_a = persist.tile([1, BL], F32, tag="se_a")
    nc.vector.reduce_sum(
        se_a[:], pex[:, 0:BL * C].rearrange("p (a b) -> p a b", b=C),
        axis=AX.X)
    se_v = persist.tile([1, BL], F32, tag="se_v")
    nc.vector.reduce_sum(
        se_v[:], pex[:, BL * C:2 * BL * C].rearrange("p (a b) -> p a b", b=C),
        axis=AX.X)

    epos = persist.tile([1, BL], F32, tag="epos")
    nc.scalar.activation(epos[:], pos_row[:], AF.Exp)
    neg_a = persist.tile([1, BL], F32, tag="neg_a")
    nc.vector.tensor_tensor(neg_a[:], se_a[:], epos[:], op=OP.subtract)
    neg_v = persist.tile([1, BL], F32, tag="neg_v")
    nc.vector.tensor_tensor(neg_v[:], se_v[:], epos[:], op=OP.subtract)
    lg_a = persist.tile([1, BL], F32, tag="lg_a")
    nc.scalar.activation(lg_a[:], neg_a[:], AF.Ln)
    lg_v = persist.tile([1, BL], F32, tag="lg_v")
    nc.scalar.activation(lg_v[:], neg_v[:], AF.Ln)
    term = persist.tile([1, BL], F32, tag="term")
    nc.vector.tensor_tensor(term[:], lg_a[:], lg_v[:], op=OP.add)
    nc.vector.tensor_scalar_mul(term[:], term[:], 0.5)
    nc.vector.tensor_tensor(term[:], term[:], pos_row[:], op=OP.subtract)
    tot = persist.tile([1, 1], F32, tag="tot")
    nc.vector.reduce_sum(tot[:], term[:], axis=AX.X)
    nc.sync.dma_start(out[:, :], tot[:])



# ---------------------------------------------------------------------------
# BIR legalization for this walrus build:
#  - it rejects instructions carrying more than one semaphore wait
#    ("Too many sync wait commands"): hoist extra waits onto single-wait
#    NoOp carriers on the same engine.
#  - the Tile tail's EVENT_SEMAPHORE_RANGE_CLEAR raw-ISA encoding mismatches
#    ("ISA wrong length"): replace with a sem-resetting Drain and drop the
#    trailing barrier that only fences the reset.
_LEGALIZE_N = [0]


def _legalize(nc):
    for fn in nc.m.functions:
        for bb in fn.blocks:
            new = []
            tail_trim = False
            for inst in bb.instructions:
                si = inst.sync_info
                if si is not None and si.on_wait and len(si.on_wait) > 1:
                    for w in list(si.on_wait[:-1]):
                        _LEGALIZE_N[0] += 1
                        new.append(mybir.InstNoOp(
                            name=f"I-lw{_LEGALIZE_N[0]}",
                            opcode="NoOp",
                            engine=inst.engine,
                            sync_info=mybir.SyncInfo(on_wait=[w],
                                                     on_update=[]),
                        ))
                    si.on_wait = [si.on_wait[-1]]
                if (isinstance(inst, mybir.InstISA)
                        and getattr(inst, "op_name", "") ==
                        "EVENT_SEMAPHORE_RANGE_CLEAR"):
                    ad = getattr(inst, "ant_dict", None) or {}
                    _LEGALIZE_N[0] += 1
                    new.append(mybir.InstDrain(
                        name=f"I-lc{_LEGALIZE_N[0]}",
                        opcode="Drain",
                        engine=inst.engine,
                        is_reset_sema=True,
                        reset_range_start=ad.get("range_first"),
                        reset_range_stop=ad.get("range_last"),
                    ))
                    tail_trim = True
                    continue
                if tail_trim and inst.opcode in ("EventSemaphore", "Drain"):
                    continue
                new.append(inst)
            bb.instructions[:] = new


_CACHE = {}


def _get_nc():
    if "nc" not in _CACHE:
        nc = bass.Bass("TRN2", target_bir_lowering=False, debug=False,
                       num_devices=NCORES)
        dt = BF16 if USE_BF16 else F32
        video = nc.dram_tensor("video", [BL, T, D], dt,
                               kind="ExternalInput").ap()
        audio = nc.dram_tensor("audio", [BL, T, D], dt,
                               kind="ExternalInput").ap()
        mask = nc.dram_tensor("mask", [BL, T], I32, kind="ExternalInput").ap()
        out = nc.dram_tensor("out", [1, 1], F32, kind="ExternalOutput").ap()
        with tile.TileContext(nc) as tc:
            with ExitStack() as ctx:
                build_kernel(ctx, tc, video, audio, mask, out)
        _legalize(nc)
        _CACHE["nc"] = nc
    return _CACHE["nc"]


def kernel(video, audio, mask, _want_results=False):
    import ml_dtypes
    ddt = ml_dtypes.bfloat16 if USE_BF16 else np.float32
    video = np.ascontiguousarray(np.asarray(video).astype(ddt))
    audio = np.ascontiguousarray(np.asarray(audio).astype(ddt))
    mask = np.ascontiguousarray(np.asarray(mask, dtype=np.int32))
    nc = _get_nc()
    in_maps = []
    for i in range(NCORES):
        sl = slice(i * BL, (i + 1) * BL)
        in_maps.append({"video": video[sl], "audio": audio[sl],
                        "mask": mask[sl]})
    res = run_bass_kernel_spmd(nc, in_maps, list(range(NCORES)))
    parts = [res.results[i]["out"][0, 0] for i in range(NCORES)]
    loss = np.float32(np.sum(np.asarray(parts, dtype=np.float64)) / B)
    outarr = np.asarray([loss], dtype=np.float32)
    if _want_results:
        return outarr, res
    return outarr
